# revision 8
# baseline (speedup 1.0000x reference)
"""Trainium2 Bass kernel for nn_Continuity_loss.

loss = -(1/R) * sum_{i in 1..2, j in 0..2} sum_{r,c} c*ln(c),
    c = |X[r,c] * X[r+i,c+j]|   (valid r,c only)

Key decomposition: |uv|ln|uv| = |u||v|(ln|u| + ln|v|).  With A = |X| and
B = A*ln(A):

    total = sum_{r,c} B[r,c] * S[r,c]
    S[r,c] = sum_{i in {1,2}, j in {0,1,2}} ( A[r+i,c+j] + A[r-i,c-j] )

(zero-padded outside the matrix; the backward offsets come from re-indexing
the A[r,c]*B[r+i,c+j] half of each term).

Per core (1024 rows + 2-row halos): stream 128-row tiles (DMA casts
f32->fp16 in flight), A = |x| clamped (DVE), L = ln A (ACT), B = A*L
(DVE), pair-sum g2[u] = A[u]+A[u+1] (DVE).  The row-shift stencil runs on
the TensorEngine with constant 0/1 shift matrices U (rows +1,+2) and V
(rows -1,-2):

    S_chunk = U.T@g2[n0+2] + U.T@a[n0+4] + V.T@g2[n0] + V.T@a[n0+2]

accumulated in PSUM.  ACT copies S to fp16, DVE forms t = B ⊙ S, and a
ones-vector matmul reduces t into a persistent [1,512] PSUM accumulator.
Final: copy + reduce + DMA out; host sums the 8 per-core partials.

Tiles overlap by 4 rows (stride 124) so every row's +-2 stencil is
in-tile; U/V zero the non-owned output rows so no row is counted twice.
fp16 intermediates (validated vs fp32: rel err ~3e-5); |x| is clamped to
>=6.5e-5 so fp16 subnormals / ln(0) never appear.
"""

from contextlib import ExitStack

import numpy as np

import concourse.bacc as bacc
import concourse.bass as bass
import concourse.tile as tile
from concourse import mybir
from concourse.bass_utils import run_bass_kernel_spmd

N_CORES = 8
R_FULL, C_FULL = 8192, 4096
ROWS_PER_CORE = R_FULL // N_CORES  # 1024
SLAB_ROWS = ROWS_PER_CORE + 4      # 1028 (2-row halo each side)
W_PAD = C_FULL + 4                 # 4100 (2-col zero pad each side)
G2_W = C_FULL + 3                  # 4099
CHUNK = 512
N_CHUNKS = C_FULL // CHUNK         # 8
CLAMP = 6.5e-5                     # > fp16 min normal; keeps ln() finite
# tile bases: 8 tiles own rows [b+2, b+126); last tile owns [994, 1026)
TILE_BASES = [124 * t for t in range(8)] + [900]
F16 = mybir.dt.float16
F32 = mybir.dt.float32
U16 = mybir.dt.uint16

_prog_cache = {}


def _shift_weights(m_lo, m_hi):
    """U[k,m]=1 iff k in {m+1,m+2}; V[k,m]=1 iff k in {m-1,m-2}; only for
    output rows m in [m_lo, m_hi) (others zeroed so the tile contributes
    exactly its owned rows)."""
    U = np.zeros((128, 128), dtype=np.float16)
    V = np.zeros((128, 128), dtype=np.float16)
    for m in range(m_lo, m_hi):
        for d in (1, 2):
            if m + d < 128:
                U[m + d, m] = 1.0
            if m - d >= 0:
                V[m - d, m] = 1.0
    return U, V


def _build_program(reps=1):
    nc = bacc.Bacc("TRN2", debug=False)
    xpad = nc.dram_tensor("xpad", [SLAB_ROWS, W_PAD], F32, kind="ExternalInput")
    out = nc.dram_tensor("out", [1, 1], F32, kind="ExternalOutput")

    U_np, V_np = _shift_weights(2, 126)
    UL_np, VL_np = _shift_weights(94, 126)
    wU_d = nc.inline_tensor(U_np, "wU")
    wV_d = nc.inline_tensor(V_np, "wV")
    wUL_d = nc.inline_tensor(UL_np, "wUL")
    wVL_d = nc.inline_tensor(VL_np, "wVL")
    wones_d = nc.inline_tensor(np.ones((128, 1), dtype=np.float16), "wones")

    with tile.TileContext(nc) as tc, ExitStack() as ctx:
        consts = ctx.enter_context(tc.tile_pool(name="consts", bufs=1))
        in_pool = ctx.enter_context(tc.tile_pool(name="in", bufs=3))
        a_pool = ctx.enter_context(tc.tile_pool(name="a", bufs=2))
        l_pool = ctx.enter_context(tc.tile_pool(name="l", bufs=2))
        b_pool = ctx.enter_context(tc.tile_pool(name="b", bufs=2))
        g_pool = ctx.enter_context(tc.tile_pool(name="g", bufs=2))
        s_pool = ctx.enter_context(tc.tile_pool(name="s", bufs=6))
        t_pool = ctx.enter_context(tc.tile_pool(name="t", bufs=6))
        ps_pool = ctx.enter_context(
            tc.tile_pool(name="ps", bufs=6, space=bass.MemorySpace.PSUM))
        acc_pool = ctx.enter_context(
            tc.tile_pool(name="acc", bufs=1, space=bass.MemorySpace.PSUM))
        fin_pool = ctx.enter_context(tc.tile_pool(name="fin", bufs=1))

        wU = consts.tile([128, 128], F16)
        wV = consts.tile([128, 128], F16)
        wUL = consts.tile([128, 128], F16)
        wVL = consts.tile([128, 128], F16)
        wones = consts.tile([128, 1], F16)
        nc.gpsimd.dma_start(wU[:], wU_d[:])
        nc.gpsimd.dma_start(wV[:], wV_d[:])
        nc.gpsimd.dma_start(wUL[:], wUL_d[:])
        nc.gpsimd.dma_start(wVL[:], wVL_d[:])
        nc.gpsimd.dma_start(wones[:], wones_d[:])

        acc = acc_pool.tile([1, CHUNK], F32)

        n_tiles = len(TILE_BASES)
        for rep in range(reps):
            for it, b in enumerate(TILE_BASES):
                u_w, v_w = (wUL, wVL) if it == n_tiles - 1 else (wU, wV)

                # load + cast f32 -> fp16 in the DMA (SWDGE)
                xt = in_pool.tile([128, W_PAD], F16, tag="xt")
                nc.gpsimd.dma_start(xt[:], xpad[b:b + 128, :])

                # A = clamp(|x|): clear sign bit, then max-clamp (clamp >
                # fp16 min normal so A is never subnormal, ln(A) finite)
                a = a_pool.tile([128, W_PAD], F16, tag="a")
                nc.vector.tensor_scalar(
                    a[:].bitcast(U16), xt[:].bitcast(U16),
                    0x7FFF, None, op0=mybir.AluOpType.bitwise_and)
                nc.vector.tensor_scalar_max(a[:], a[:], CLAMP)

                # L = ln(A), B = A*L on the interior 4096 cols
                l = l_pool.tile([128, C_FULL], F16, tag="l")
                nc.scalar.activation(
                    l[:], a[:, 2:2 + C_FULL], mybir.ActivationFunctionType.Ln)
                bb = b_pool.tile([128, C_FULL], F16, tag="bb")
                nc.vector.tensor_mul(bb[:], a[:, 2:2 + C_FULL], l[:])

                # g2[u] = a[u] + a[u+1]
                g2 = g_pool.tile([128, G2_W], F16, tag="g2")
                nc.vector.tensor_add(g2[:], a[:, 0:G2_W], a[:, 1:1 + G2_W])

                for k in range(N_CHUNKS):
                    n0 = k * CHUNK
                    ps = ps_pool.tile([128, CHUNK], F32, tag="ps")
                    nc.tensor.matmul(ps[:], u_w[:], g2[:, n0 + 2:n0 + 2 + CHUNK],
                                     start=True, stop=False)
                    nc.tensor.matmul(ps[:], u_w[:], a[:, n0 + 4:n0 + 4 + CHUNK],
                                     start=False, stop=False)
                    nc.tensor.matmul(ps[:], v_w[:], g2[:, n0:n0 + CHUNK],
                                     start=False, stop=False)
                    nc.tensor.matmul(ps[:], v_w[:], a[:, n0 + 2:n0 + 2 + CHUNK],
                                     start=False, stop=True)
                    s16 = s_pool.tile([128, CHUNK], F16, tag="s16")
                    nc.scalar.copy(s16[:], ps[:])
                    t16 = t_pool.tile([128, CHUNK], F16, tag="t16")
                    nc.vector.tensor_mul(t16[:], bb[:, n0:n0 + CHUNK], s16[:])
                    nc.tensor.matmul(
                        acc[:], wones[:], t16[:],
                        start=(rep == 0 and it == 0 and k == 0),
                        stop=(rep == reps - 1 and it == n_tiles - 1
                              and k == N_CHUNKS - 1))

        acc_s = fin_pool.tile([1, CHUNK], F32)
        nc.vector.tensor_copy(acc_s[:], acc[:])
        res = fin_pool.tile([1, 1], F32)
        nc.vector.reduce_sum(res[:], acc_s[:], axis=mybir.AxisListType.X)
        nc.gpsimd.dma_start(out[:], res[:])

    nc.compile()
    return nc


def kernel(X, neighbor):
    assert int(neighbor) == 3
    X = np.ascontiguousarray(np.asarray(X, dtype=np.float32))
    assert X.shape == (R_FULL, C_FULL)

    if "nc" not in _prog_cache:
        _prog_cache["nc"] = _build_program()
    nc = _prog_cache["nc"]

    Xp = np.pad(X, ((2, 2), (2, 2)))
    in_maps = [
        {"xpad": Xp[m * ROWS_PER_CORE:m * ROWS_PER_CORE + SLAB_ROWS, :]}
        for m in range(N_CORES)
    ]
    res = run_bass_kernel_spmd(nc, in_maps, list(range(N_CORES)))
    total = sum(float(r["out"][0, 0]) for r in res.results)
    return np.array([-total / R_FULL], dtype=np.float32)


# revision 10
# speedup vs baseline: 5.9066x; 5.9066x over previous
"""Trainium2 Bass kernel for nn_Continuity_loss.

loss = -(1/R) * sum_{i in 1..2, j in 0..2} sum_{r,c} c*ln(c),
    c = |X[r,c] * X[r+i,c+j]|   (valid r,c only)

Key decomposition: |uv|ln|uv| = |u||v|(ln|u| + ln|v|).  With A = |X| and
B = A*ln(A):

    total = sum_{r,c} B[r,c] * S[r,c]
    S[r,c] = sum_{i in {1,2}, j in {0,1,2}} ( A[r+i,c+j] + A[r-i,c-j] )

(zero-padded outside the matrix; the backward offsets come from re-indexing
the A[r,c]*B[r+i,c+j] half of each term).

Per core (1024 rows + 2-row halos): stream 128-row tiles (DMA casts
f32->fp16 in flight), A = |x| clamped (DVE), L = ln A (ACT), B = A*L
(DVE), pair-sum g2[u] = A[u]+A[u+1] (DVE).  The row-shift stencil runs on
the TensorEngine with constant 0/1 shift matrices U (rows +1,+2) and V
(rows -1,-2):

    S_chunk = U.T@g2[n0+2] + U.T@a[n0+4] + V.T@g2[n0] + V.T@a[n0+2]

accumulated in PSUM.  ACT copies S to fp16, DVE forms t = B ⊙ S, and a
ones-vector matmul reduces t into a persistent [1,512] PSUM accumulator.
Final: copy + reduce + DMA out; host sums the 8 per-core partials.

Tiles overlap by 4 rows (stride 124) so every row's +-2 stencil is
in-tile; U/V zero the non-owned output rows so no row is counted twice.
fp16 intermediates (validated vs fp32: rel err ~3e-5); |x| is clamped to
>=6.5e-5 so fp16 subnormals / ln(0) never appear.
"""

from contextlib import ExitStack

import numpy as np

import concourse.bacc as bacc
import concourse.bass as bass
import concourse.tile as tile
from concourse import mybir
from concourse.bass_utils import run_bass_kernel_spmd

N_CORES = 8
R_FULL, C_FULL = 8192, 4096
ROWS_PER_CORE = R_FULL // N_CORES  # 1024
SLAB_ROWS = ROWS_PER_CORE + 4      # 1028 (2-row halo each side)
W_PAD = C_FULL + 4                 # 4100 (2-col zero pad each side)
G2_W = C_FULL + 3                  # 4099
CHUNK = 512
N_CHUNKS = C_FULL // CHUNK         # 8
CLAMP = 6.5e-5                     # > fp16 min normal; keeps ln() finite
# tile bases: 8 tiles own rows [b+2, b+126); last tile owns [994, 1026)
TILE_BASES = [124 * t for t in range(8)] + [900]
F16 = mybir.dt.float16
F32 = mybir.dt.float32
U16 = mybir.dt.uint16

_prog_cache = {}


def _shift_weights(m_lo, m_hi):
    """U[k,m]=1 iff k in {m+1,m+2}; V[k,m]=1 iff k in {m-1,m-2}; only for
    output rows m in [m_lo, m_hi) (others zeroed so the tile contributes
    exactly its owned rows)."""
    U = np.zeros((128, 128), dtype=np.float16)
    V = np.zeros((128, 128), dtype=np.float16)
    for m in range(m_lo, m_hi):
        for d in (1, 2):
            if m + d < 128:
                U[m + d, m] = 1.0
            if m - d >= 0:
                V[m - d, m] = 1.0
    return U, V


def _build_program(reps=1, cast_dma=True):
    nc = bacc.Bacc("TRN2", debug=False)
    xpad = nc.dram_tensor("xpad", [SLAB_ROWS, W_PAD], F32, kind="ExternalInput")
    out = nc.dram_tensor("out", [1, 1], F32, kind="ExternalOutput")

    U_np, V_np = _shift_weights(2, 126)
    UL_np, VL_np = _shift_weights(94, 126)
    wU_d = nc.inline_tensor(U_np, "wU")
    wV_d = nc.inline_tensor(V_np, "wV")
    wUL_d = nc.inline_tensor(UL_np, "wUL")
    wVL_d = nc.inline_tensor(VL_np, "wVL")
    wones_d = nc.inline_tensor(np.ones((128, 1), dtype=np.float16), "wones")

    with tile.TileContext(nc) as tc, ExitStack() as ctx:
        consts = ctx.enter_context(tc.tile_pool(name="consts", bufs=1))
        in_pool = ctx.enter_context(tc.tile_pool(name="in", bufs=3))
        a_pool = ctx.enter_context(tc.tile_pool(name="a", bufs=2))
        l_pool = ctx.enter_context(tc.tile_pool(name="l", bufs=2))
        b_pool = ctx.enter_context(tc.tile_pool(name="b", bufs=2))
        g_pool = ctx.enter_context(tc.tile_pool(name="g", bufs=2))
        s_pool = ctx.enter_context(tc.tile_pool(name="s", bufs=6))
        t_pool = ctx.enter_context(tc.tile_pool(name="t", bufs=6))
        ps_pool = ctx.enter_context(
            tc.tile_pool(name="ps", bufs=6, space=bass.MemorySpace.PSUM))
        acc_pool = ctx.enter_context(
            tc.tile_pool(name="acc", bufs=1, space=bass.MemorySpace.PSUM))
        fin_pool = ctx.enter_context(tc.tile_pool(name="fin", bufs=1))

        wU = consts.tile([128, 128], F16)
        wV = consts.tile([128, 128], F16)
        wUL = consts.tile([128, 128], F16)
        wVL = consts.tile([128, 128], F16)
        wones = consts.tile([128, 1], F16)
        nc.gpsimd.dma_start(wU[:], wU_d[:])
        nc.gpsimd.dma_start(wV[:], wV_d[:])
        nc.gpsimd.dma_start(wUL[:], wUL_d[:])
        nc.gpsimd.dma_start(wVL[:], wVL_d[:])
        nc.gpsimd.dma_start(wones[:], wones_d[:])

        acc = acc_pool.tile([1, CHUNK], F32)

        n_tiles = len(TILE_BASES)
        for rep in range(reps):
            for it, b in enumerate(TILE_BASES):
                u_w, v_w = (wUL, wVL) if it == n_tiles - 1 else (wU, wV)

                a = a_pool.tile([128, W_PAD], F16, tag="a")
                if cast_dma:
                    # load + cast f32 -> fp16 in the DMA (SWDGE)
                    xt = in_pool.tile([128, W_PAD], F16, tag="xt")
                    nc.gpsimd.dma_start(xt[:], xpad[b:b + 128, :])
                    # A = clamp(|x|): clear sign bit, then max-clamp (clamp
                    # > fp16 min normal: A never subnormal, ln(A) finite)
                    nc.vector.tensor_scalar(
                        a[:].bitcast(U16), xt[:].bitcast(U16),
                        0x7FFF, None, op0=mybir.AluOpType.bitwise_and)
                    nc.vector.tensor_scalar_max(a[:], a[:], CLAMP)
                else:
                    xt = in_pool.tile([128, W_PAD], F32, tag="xt")
                    nc.gpsimd.dma_start(xt[:], xpad[b:b + 128, :])
                    nc.vector.tensor_scalar(
                        xt[:].bitcast(mybir.dt.uint32),
                        xt[:].bitcast(mybir.dt.uint32),
                        0x7FFFFFFF, None, op0=mybir.AluOpType.bitwise_and)
                    nc.vector.tensor_scalar_max(a[:], xt[:], CLAMP)

                # L = ln(A), B = A*L on the interior 4096 cols
                l = l_pool.tile([128, C_FULL], F16, tag="l")
                nc.scalar.activation(
                    l[:], a[:, 2:2 + C_FULL], mybir.ActivationFunctionType.Ln)
                bb = b_pool.tile([128, C_FULL], F16, tag="bb")
                nc.vector.tensor_mul(bb[:], a[:, 2:2 + C_FULL], l[:])

                # g2[u] = a[u] + a[u+1]
                g2 = g_pool.tile([128, G2_W], F16, tag="g2")
                nc.vector.tensor_add(g2[:], a[:, 0:G2_W], a[:, 1:1 + G2_W])

                for k in range(N_CHUNKS):
                    n0 = k * CHUNK
                    ps = ps_pool.tile([128, CHUNK], F32, tag="ps")
                    nc.tensor.matmul(ps[:], u_w[:], g2[:, n0 + 2:n0 + 2 + CHUNK],
                                     start=True, stop=False)
                    nc.tensor.matmul(ps[:], u_w[:], a[:, n0 + 4:n0 + 4 + CHUNK],
                                     start=False, stop=False)
                    nc.tensor.matmul(ps[:], v_w[:], g2[:, n0:n0 + CHUNK],
                                     start=False, stop=False)
                    nc.tensor.matmul(ps[:], v_w[:], a[:, n0 + 2:n0 + 2 + CHUNK],
                                     start=False, stop=True)
                    s16 = s_pool.tile([128, CHUNK], F16, tag="s16")
                    nc.scalar.copy(s16[:], ps[:])
                    t16 = t_pool.tile([128, CHUNK], F16, tag="t16")
                    nc.vector.tensor_mul(t16[:], bb[:, n0:n0 + CHUNK], s16[:])
                    nc.tensor.matmul(
                        acc[:], wones[:], t16[:],
                        start=(rep == 0 and it == 0 and k == 0),
                        stop=(rep == reps - 1 and it == n_tiles - 1
                              and k == N_CHUNKS - 1))

        acc_s = fin_pool.tile([1, CHUNK], F32)
        nc.vector.tensor_copy(acc_s[:], acc[:])
        res = fin_pool.tile([1, 1], F32)
        nc.vector.reduce_sum(res[:], acc_s[:], axis=mybir.AxisListType.X)
        nc.gpsimd.dma_start(out[:], res[:])

    nc.compile()
    return nc


def kernel(X, neighbor):
    assert int(neighbor) == 3
    X = np.ascontiguousarray(np.asarray(X, dtype=np.float32))
    assert X.shape == (R_FULL, C_FULL)

    if "nc" not in _prog_cache:
        _prog_cache["nc"] = _build_program()
    nc = _prog_cache["nc"]

    Xp = np.pad(X, ((2, 2), (2, 2)))
    in_maps = [
        {"xpad": Xp[m * ROWS_PER_CORE:m * ROWS_PER_CORE + SLAB_ROWS, :]}
        for m in range(N_CORES)
    ]
    res = run_bass_kernel_spmd(nc, in_maps, list(range(N_CORES)))
    total = sum(float(r["out"][0, 0]) for r in res.results)
    return np.array([-total / R_FULL], dtype=np.float32)
